# revision 1
# baseline (speedup 1.0000x reference)
"""BMMRemapper Trainium2 kernel.

Math: out[n,c,q] = sum_k x[n,c,k] * mat[n,q,k] where mat is the bilinear
interpolation matrix built from grid (4 nonzeros per row q: rows lin, lin+1,
lin+48, lin+49 of x^T with weights (1-a)(1-b), (1-a)b, a(1-b), ab).

Instead of a dense 2304x2304 BMM we exploit the 4-sparsity: the host stages
a quad-row table xq[k] = [x^T[k], x^T[k+1], x^T[k+48], x^T[k+49]] (pure data
movement), so ONE indirect-DMA descriptor per output pixel fetches all four
corner rows (2 KB contiguous). 18 gathers of [128, 512] cover all 2304
pixels; a per-tile scalar_tensor_tensor chain applies the bilinear weights
(per-partition scalars) and accumulates.

Sharding: batch-parallel, one batch per NeuronCore (N=8 = n_cores), no
cross-core communication. The disk mask couples batches (all-batch AND), so
every core receives the full grid (tiny) and computes the mask locally.

Layouts (q = output pixel, 0..2303; t = q//128; p = q%128):
  xq     (2304, 512) f32 : quad-row table (row k -> 4 corner rows for lin=k).
  gcoef  (128, 36)   f32 : own-batch grid, [p, 2*t+coord].
  gall   (128, 288)  f32 : all-batch grid, [p, 16*t + 2*m + coord].
  outp   (128, 2304) f32 : [p, t*128 + c]  (host re-permutes to (c, q)).
"""

import numpy as np

N, H, W, C = 8, 48, 48, 128
HW = H * W            # 2304
NT = HW // 128        # 18
EPS = 1e-5
CLIP_HI = float(np.float32(float(H - 1) - EPS))  # 46.99999 (f32)

_CACHE = {}


def _build_nc():
    from contextlib import ExitStack

    import concourse.bacc as bacc
    import concourse.bass as bass
    import concourse.mybir as mybir
    import concourse.tile as tile

    dt = mybir.dt
    f32, i32 = dt.float32, dt.int32
    Alu = mybir.AluOpType

    nc = bacc.Bacc("TRN2", target_bir_lowering=False, debug=False, num_devices=N)

    xq = nc.dram_tensor("xq", [HW, 4 * C], f32, kind="ExternalInput")
    gcoef = nc.dram_tensor("gcoef", [128, 2 * NT], f32, kind="ExternalInput")
    gall = nc.dram_tensor("gall", [128, 16 * NT], f32, kind="ExternalInput")
    outp = nc.dram_tensor("outp", [128, HW], f32, kind="ExternalOutput")

    with tile.TileContext(nc) as tc, ExitStack() as ctx:
        pool = ctx.enter_context(tc.tile_pool(name="p", bufs=1))

        # ---- load grid layouts (HWDGE) ----
        g_coef = pool.tile([128, 2 * NT], f32)
        g_all = pool.tile([128, 16 * NT], f32)
        nc.sync.dma_start(g_coef[:], gcoef.ap())
        nc.sync.dma_start(g_all[:], gall.ap())

        # floor(x): int cast rounds-to-nearest on HW (truncates in CoreSim);
        # correct with "subtract 1 where cast > x" which is exact for both.
        _flr = [0]

        def floor_f32(src, n_cols, eng):
            k = _flr[0]
            _flr[0] += 1
            ti = pool.tile([128, n_cols], i32, tag=f"flr_i{k}")
            eng.tensor_copy(ti[:], src)
            tf = pool.tile([128, n_cols], f32, tag=f"flr_f{k}")
            eng.tensor_copy(tf[:], ti[:])
            gt = pool.tile([128, n_cols], f32, tag=f"flr_g{k}")
            eng.tensor_tensor(gt[:], tf[:], src, Alu.is_gt)
            out = pool.tile([128, n_cols], f32, tag=f"flr_o{k}")
            eng.tensor_tensor(out[:], tf[:], gt[:], Alu.subtract)
            return out

        # ---- clip + floor ([128, NT]; q = t*128 + p) ----
        # a-coord chain on DVE, b-coord chain on GPSIMD: halves the serial
        # latency on the gather-index critical path.
        ca = pool.tile([128, NT], f32)
        cb = pool.tile([128, NT], f32)
        nc.vector.tensor_scalar(ca[:], g_coef[:, 0::2], EPS, CLIP_HI, Alu.max, Alu.min)
        nc.vector.tensor_scalar(cb[:], g_coef[:, 1::2], EPS, CLIP_HI, Alu.max, Alu.min)
        ba2f = floor_f32(ca[:], NT, nc.vector)
        bb2f = floor_f32(cb[:], NT, nc.vector)

        # ---- gather indices: lin = floor(a)*W + floor(b) ----
        linf = pool.tile([128, NT], f32)
        nc.vector.scalar_tensor_tensor(
            linf[:], ba2f[:], float(W), bb2f[:], Alu.mult, Alu.add
        )
        idx = pool.tile([128, NT], i32)
        nc.vector.tensor_copy(idx[:], linf[:])

        # ---- indirect quad gathers (one standalone dest tile per t) ----
        gts = []
        for t in range(NT):
            gt_t = pool.tile([128, 4 * C], f32, tag=f"G{t}")
            nc.gpsimd.indirect_dma_start(
                out=gt_t[:],
                out_offset=None,
                in_=xq.ap(),
                in_offset=bass.IndirectOffsetOnAxis(ap=idx[:, t : t + 1], axis=0),
            )
            gts.append(gt_t)

        # ---- mask ([128, NT]): AND over all batches of in-bounds test ----
        g_all3 = g_all[:].rearrange("p (t m) -> p t m", m=16)
        mn = pool.tile([128, NT], f32)
        mx = pool.tile([128, NT], f32)
        nc.vector.tensor_reduce(mn[:], g_all3, mybir.AxisListType.X, Alu.min)
        nc.vector.tensor_reduce(mx[:], g_all3, mybir.AxisListType.X, Alu.max)
        mge = pool.tile([128, NT], f32)
        mle = pool.tile([128, NT], f32)
        nc.vector.tensor_scalar(mge[:], mn[:], -0.5, None, Alu.is_ge)
        nc.vector.tensor_scalar(mle[:], mx[:], float(H) - 0.5, None, Alu.is_le)
        mask = pool.tile([128, NT], f32)
        nc.vector.tensor_tensor(mask[:], mge[:], mle[:], Alu.mult)

        # ---- coefficients ([128, NT]) ----
        fa = pool.tile([128, NT], f32)   # a  (row frac)
        fb = pool.tile([128, NT], f32)   # b  (col frac)
        nc.vector.tensor_tensor(fa[:], ca[:], ba2f[:], Alu.subtract)
        nc.vector.tensor_tensor(fb[:], cb[:], bb2f[:], Alu.subtract)
        fb0 = pool.tile([128, NT], f32)   # 1-b
        nc.vector.tensor_scalar(fb0[:], fb[:], -1.0, 1.0, Alu.mult, Alu.add)
        fa0 = pool.tile([128, NT], f32)   # 1-a
        nc.vector.tensor_scalar(fa0[:], fa[:], -1.0, 1.0, Alu.mult, Alu.add)
        fa0m = pool.tile([128, NT], f32)  # (1-a)*mask
        fa1m = pool.tile([128, NT], f32)  # a*mask
        nc.vector.tensor_tensor(fa0m[:], fa0[:], mask[:], Alu.mult)
        nc.vector.tensor_tensor(fa1m[:], fa[:], mask[:], Alu.mult)

        c00 = pool.tile([128, NT], f32)
        c01 = pool.tile([128, NT], f32)
        c10 = pool.tile([128, NT], f32)
        c11 = pool.tile([128, NT], f32)
        nc.vector.tensor_tensor(c00[:], fa0m[:], fb0[:], Alu.mult)
        nc.vector.tensor_tensor(c01[:], fa0m[:], fb[:], Alu.mult)
        nc.vector.tensor_tensor(c10[:], fa1m[:], fb0[:], Alu.mult)
        nc.vector.tensor_tensor(c11[:], fa1m[:], fb[:], Alu.mult)

        # ---- combine per tile: out_t = c00*A + c01*B + c10*Cr + c11*D ----
        out_sb = pool.tile([128, HW], f32)
        for t in range(NT):
            g = gts[t]
            A = g[:, 0 * C : 1 * C]
            B = g[:, 1 * C : 2 * C]
            Cr = g[:, 2 * C : 3 * C]
            D = g[:, 3 * C : 4 * C]
            eng = nc.vector
            # two products on the otherwise-idle ACT engine (per-partition
            # scale); DVE does the other two (fused mult-add) plus one add.
            u0 = pool.tile([128, C], f32, tag=f"u0_{t}")
            u1 = pool.tile([128, C], f32, tag=f"u1_{t}")
            nc.scalar.activation(
                u0[:], A, mybir.ActivationFunctionType.Copy,
                scale=c00[:, t : t + 1],
            )
            nc.scalar.activation(
                u1[:], B, mybir.ActivationFunctionType.Copy,
                scale=c01[:, t : t + 1],
            )
            v0 = pool.tile([128, C], f32, tag=f"v0_{t}")
            eng.scalar_tensor_tensor(
                v0[:], Cr, c10[:, t : t + 1], u0[:], Alu.mult, Alu.add
            )
            v1 = pool.tile([128, C], f32, tag=f"v1_{t}")
            eng.scalar_tensor_tensor(
                v1[:], D, c11[:, t : t + 1], u1[:], Alu.mult, Alu.add
            )
            eng.tensor_tensor(
                out_sb[:, t * C : (t + 1) * C], v0[:], v1[:], Alu.add
            )

        # ---- store (chunked so early tiles stream out under later work) ----
        for c0 in range(0, NT, 6):
            c1 = min(c0 + 6, NT)
            nc.sync.dma_start(
                outp.ap()[:, c0 * C : c1 * C], out_sb[:, c0 * C : c1 * C]
            )

    nc.compile()
    return nc


def _get_nc():
    if "nc" not in _CACHE:
        _CACHE["nc"] = _build_nc()
    return _CACHE["nc"]


def _stage_inputs(x, grid):
    """Build the per-core input maps (pure data movement / replication)."""
    x = np.ascontiguousarray(x, dtype=np.float32)
    grid = np.ascontiguousarray(grid, dtype=np.float32)
    xr = x.reshape(N, C, HW)
    gr = grid.reshape(N, HW, 2)

    # quad-row table: xq[n][k] = [xT[k], xT[k+1], xT[k+48], xT[k+49]]
    xt = np.zeros((N, HW + W + 2, C), dtype=np.float32)
    xt[:, :HW] = xr.transpose(0, 2, 1)
    xq = np.empty((N, HW, 4 * C), dtype=np.float32)
    xq[:, :, 0 * C : 1 * C] = xt[:, 0 : HW]
    xq[:, :, 1 * C : 2 * C] = xt[:, 1 : HW + 1]
    xq[:, :, 2 * C : 3 * C] = xt[:, W : HW + W]
    xq[:, :, 3 * C : 4 * C] = xt[:, W + 1 : HW + W + 1]

    # gcoef[n][p, 2t+c] = gr[n, t*128+p, c]
    gc = gr.reshape(N, NT, 128, 2).transpose(0, 2, 1, 3)  # [n, p, t, c]
    gcoef = np.ascontiguousarray(gc.reshape(N, 128, 2 * NT))

    # gall[p, 16t+2m+c] = gr[m, t*128+p, c]   (same for all cores)
    ga = gr.reshape(N, NT, 128, 2).transpose(2, 1, 0, 3)  # [p, t, m, c]
    gall = np.ascontiguousarray(ga.reshape(128, 16 * NT))

    return [{"xq": xq[n], "gcoef": gcoef[n], "gall": gall} for n in range(N)]


def _unstage_output(results):
    """results[n]["outp"] is (128, 2304) = [p, t*128+c] -> (N, C, H, W)."""
    out = np.empty((N, C, H, W), dtype=np.float32)
    for n in range(N):
        o = results[n]["outp"].reshape(128, NT, C)       # [p, t, c]
        out[n] = o.transpose(2, 1, 0).reshape(C, H, W)   # [c, q=t*128+p]
    return out


def kernel(x, grid):
    from concourse import bass_utils

    nc = _get_nc()
    in_maps = _stage_inputs(x, grid)
    res = bass_utils.run_bass_kernel_spmd(nc, in_maps, core_ids=list(range(N)))
    return _unstage_output(res.results)

